# revision 3
# baseline (speedup 1.0000x reference)
"""LMS adaptive filter (BaseFilter) on 8 TRN2 NeuronCores — v5.

Algorithm per (batch b, frame f): 64-tap LMS over 416 sequential steps.
  e_t   = d[b, 256f + 32 + t] - sum_k w[k] * x[256f + t + k]
  w     = clip(w + MU * e_t * x[256f + t : +64], +-65535)
The clip is essential (mu*|x_win|^2 ~ 3.2 makes the recursion unstable;
w rides the rails, which keeps float implementations shadowing).

Sharding: 4096 frames split 512/core (both batches on every core) ->
1024 sequences/core = 2 b-halves (phase chains) x 4 slabs x 128 parts.

Per step, per b-half:
  4x DVE stt+accum  : dot_g = sum(W_g * Xwin_g); readacc -> EOUT[:,g,t]
                      (EOUT collects -ns; e/d_est assembled post-loop)
  GpSimd TT (FD=4)  : ETILE = D_t + (-ns)   (the update scalars e)
  GpSimd TT (FD=256): T2 = Ebcast (0-stride) * MU*Xwin
  DVE custom op     : W = clip(W + T2, +-65535)   (fused add+clip)
"""

import numpy as np

HOP = 256
FRAMELEN = 512
K = 64
WD = 32
MU = 0.05
WMIN, WMAX = -65535.0, 65535.0
B = 2
F = 4096
NC = 8
F_LOC = F // NC              # 512 frames per core
S = (FRAMELEN - K) - WD      # 416 sequential steps
TSTART = (FRAMELEN - HOP) - WD  # 224: first step kept for frames >= 1
TAIL = S - TSTART            # 192 output elements per frame >= 1
SPAN = HOP * (F_LOC - 1) + FRAMELEN  # 131328: x/d elements per core shard
CORE_STRIDE = HOP * F_LOC    # 131072
OUT_LEN = (FRAMELEN - K) + (F - 1) * TAIL  # 786688

# engine for T2 = e*mu*x per b-half: 'g' GpSimd, 'v' DVE
T2_ENGINE = "g"
# engine for e batch op per b-half: 'g' GpSimd, 'v' DVE
E_ENGINE = "g"
# addclip: 'c' = fused custom DVE op, 's' = DVE TT add + GpSimd clip
ADDCLIP = "c"

_CACHE = {}


def _addclip_op():
    """Custom DVE op: out = clip(in0 + in1, imm2, s1)."""
    import concourse.dve_ops as dve_ops
    from concourse.dve_ops import DveOp
    from concourse.dve_spec import (
        Spec, Src0, Src1, C1, C2, maxx, minn, lower, _has_src1,
    )
    from concourse.dve_uop import DveOpSpec

    name = "LMS_ADD_CLIP"
    for op in dve_ops.OPS:
        if op.name == name:
            return op

    def _ref(in0, in1, s0, s1, imm2):
        v = in0.astype(np.float32) + in1.astype(np.float32)
        return np.clip(v, imm2, s1).astype(np.float32)

    spec = Spec(body=maxx(minn(Src0 + Src1, C1), C2), reference=_ref)
    shas = {}
    for ver in ("v3", "v4"):
        tmp = DveOpSpec(name=name, opcode=1, uops=lower(spec, ver=ver),
                        rd1_en=_has_src1(spec))
        shas[ver] = tmp.sha(ver)
    op = DveOp(name, spec, subdim=False, uops_sha=shas)
    dve_ops.OPS.append(op)
    dve_ops.CUSTOM_DVE_SPECS[name] = spec
    dve_ops._SUB_OPCODE_FOR_NAME[name] = (
        dve_ops._CUSTOM_DVE_ROW_BASE + len(dve_ops.OPS) - 1)
    return op


def _build():
    import concourse.bacc as bacc
    import concourse.tile as tile
    from concourse import mybir
    import concourse.bass as bass

    f32 = mybir.dt.float32
    AluOp = mybir.AluOpType
    addclip = _addclip_op() if ADDCLIP == "c" else None

    nc = bacc.Bacc("TRN2", target_bir_lowering=False)
    x_in = nc.dram_tensor("x", [SPAN], f32, kind="ExternalInput")
    d_in = nc.dram_tensor("d", [B, SPAN], f32, kind="ExternalInput")
    out_main = nc.dram_tensor("out_main", [2, B, F_LOC, TAIL], f32,
                              kind="ExternalOutput")
    out_head = nc.dram_tensor("out_head", [2, B, TSTART], f32,
                              kind="ExternalOutput")

    def ap3(tile_ap, offset, dims):
        return bass.AP(tensor=tile_ap.tensor, offset=tile_ap.offset + offset,
                       ap=[tile_ap.ap[0]] + dims)

    with tile.TileContext(nc) as tc:
        with tc.tile_pool(name="p", bufs=1) as pool:
            XF = pool.tile([128, 4, FRAMELEN], f32)
            XFMU = pool.tile([128, 4, FRAMELEN], f32)
            DB = pool.tile([128, B, 4, S], f32)
            W = pool.tile([128, B, 4, K], f32)
            EOUT = pool.tile([128, B, 4, S], f32)   # -ns, then e post-loop
            DEST = pool.tile([128, B, 4, S], f32)
            ET = pool.tile([128, B, 4], f32)        # e scratch per step
            T2 = pool.tile([128, B, 4, K], f32)     # e * mu * x
            PROD = [pool.tile([128, K], f32, name=f"PROD{g}", tag=f"p{g}")
                    for g in range(8)]

            nc.sync.dma_start(
                XF[:],
                bass.AP(tensor=x_in, offset=0,
                        ap=[[HOP, 128], [HOP * 128, 4], [1, FRAMELEN]]),
            )
            for b in range(B):
                nc.sync.dma_start(
                    DB[:, b, :, :],
                    bass.AP(tensor=d_in, offset=b * SPAN + WD,
                            ap=[[HOP, 128], [HOP * 128, 4], [1, S]]),
                )
            nc.vector.tensor_scalar_mul(XFMU[:], XF[:], MU)
            nc.vector.memset(W[:], 0.0)

            for t in range(S):
                for b in range(B):
                    for fg in range(4):
                        g = b * 4 + fg
                        nc.vector.scalar_tensor_tensor(
                            out=PROD[g][:], in0=W[:, b, fg, :], scalar=-1.0,
                            in1=XF[:, fg, t:t + K],
                            op0=AluOp.mult, op1=AluOp.mult,
                            accum_out=ap3(EOUT[:], g * S + t, [[1, 1]]),
                        )
                    # e = d + (-ns)   [FD=4, strided]
                    eng_e = nc.gpsimd if E_ENGINE == "g" else nc.vector
                    eng_e.tensor_tensor(
                        out=ET[:, b, :],
                        in0=ap3(DB[:], (b * 4) * S + t, [[S, 4]]),
                        in1=ap3(EOUT[:], (b * 4) * S + t, [[S, 4]]),
                        op=AluOp.add)
                    # T2 = e_bcast * (MU*x windows)   [FD=256]
                    eng_t2 = nc.gpsimd if T2_ENGINE == "g" else nc.vector
                    eng_t2.tensor_tensor(
                        out=T2[:, b, :, :],
                        in0=ap3(XFMU[:], t, [[FRAMELEN, 4], [1, K]]),
                        in1=ap3(ET[:], b * 4, [[1, 4], [0, K]]),
                        op=AluOp.mult)
                    # W = clip(W + T2)
                    whalf = ap3(W[:], b * 4 * K, [[1, 4 * K]])
                    t2half = ap3(T2[:], b * 4 * K, [[1, 4 * K]])
                    if ADDCLIP == "c":
                        nc.vector._custom_dve(
                            addclip, out=whalf, in0=whalf, in1=t2half,
                            s1=WMAX, imm2=WMIN)
                    else:
                        nc.vector.tensor_tensor(out=whalf, in0=whalf,
                                                in1=t2half, op=AluOp.add)
                        nc.gpsimd.tensor_scalar(
                            out=whalf, in0=whalf,
                            scalar1=WMAX, scalar2=WMIN,
                            op0=AluOp.min, op1=AluOp.max)

            # post-loop: EOUT holds -ns; e = d + (-ns); d_est = d - e
            for b in range(B):
                eb = ap3(EOUT[:], b * 4 * S, [[1, 4 * S]])
                db = ap3(DB[:], b * 4 * S, [[1, 4 * S]])
                de = ap3(DEST[:], b * 4 * S, [[1, 4 * S]])
                nc.vector.tensor_tensor(out=eb, in0=db, in1=eb, op=AluOp.add)
                nc.vector.tensor_tensor(out=de, in0=db, in1=eb,
                                        op=AluOp.subtract)

            for kind, src in ((0, DEST), (1, EOUT)):
                for b in range(B):
                    for fg in range(4):
                        nc.sync.dma_start(
                            bass.AP(tensor=out_main,
                                    offset=(kind * B + b) * F_LOC * TAIL
                                    + fg * 128 * TAIL,
                                    ap=[[TAIL, 128], [1, TAIL]]),
                            src[:, b, fg, TSTART:S],
                        )
            for kind, src in ((0, DEST), (1, EOUT)):
                for b in range(B):
                    nc.sync.dma_start(
                        bass.AP(tensor=out_head,
                                offset=(kind * B + b) * TSTART,
                                ap=[[TSTART, 1], [1, TSTART]]),
                        src[0:1, b, 0, 0:TSTART],
                    )
    nc.finalize()
    return nc


def _get_nc():
    if "nc" not in _CACHE:
        _CACHE["nc"] = _build()
    return _CACHE["nc"]


def run_shards(d, x, trace=False, **kw):
    from concourse.bass_utils import run_bass_kernel_spmd

    nc = _get_nc()
    in_maps = []
    for c in range(NC):
        lo = c * CORE_STRIDE
        in_maps.append({
            "x": np.ascontiguousarray(x[lo:lo + SPAN], dtype=np.float32),
            "d": np.ascontiguousarray(d[:, lo:lo + SPAN], dtype=np.float32),
        })
    return run_bass_kernel_spmd(nc, in_maps, core_ids=list(range(NC)),
                                trace=trace, **kw)


def assemble(results):
    mains = np.stack([r["out_main"] for r in results])  # (8, 2, B, 512, 192)
    head = results[0]["out_head"]                       # (2, B, 224)
    outs = []
    for kind in range(2):
        m = mains[:, kind].transpose(1, 0, 2, 3).reshape(B, F, TAIL)
        o = np.zeros((B, OUT_LEN), np.float32)
        o[:, WD:WD + TSTART] = head[kind]
        o[:, WD + TSTART:FRAMELEN - K] = m[:, 0]
        o[:, FRAMELEN - K:] = m[:, 1:].reshape(B, -1)
        outs.append(o)
    return outs[0], outs[1]


def kernel(d, x):
    res = run_shards(d, x)
    return assemble(res.results)
